# revision 1
# baseline (speedup 1.0000x reference)
"""Conditional-layers (MoE routing) kernel for Trainium2, 8 NeuronCores.

Reference: out[b] = W[condition[b]].T @ x[b]  (einsum 'bct,bcd->bdt')
  x:         [64, 256, 4096] f32
  weights:   [16, 256, 256]  f32
  condition: [64]            int64
  out:       [64, 256, 4096] f32

Sharding: data-parallel over batch — each of the 8 cores handles 8 samples.
The per-sample weight gather (the routing) is done host-side: the gathered
[8, 256, 256] table is the per-core weight input, so the compiled program is
condition-independent (condition only changes input *data*, never the NEFF).

Per core, per sample: y[b] (256x4096) = w[b].T (256x256) @ x[b] (256x4096),
tiled as 2 K-chunks (contraction, PSUM-accumulated) x 2 D-chunks (output
partitions) x 8 T-chunks of 512 (PSUM bank = 512 f32).
"""

import numpy as np

import concourse.mybir as mybir
import concourse.tile as tile
from concourse import bacc
from concourse.bass_utils import run_bass_kernel_spmd

F32 = mybir.dt.float32

N_CORES = 8
B = 64
BS = B // N_CORES  # batch shard per core
C = 256            # in channels (contraction)
D = 256            # out channels
T = 4096
P = 128            # partition dim
TN = 512           # PSUM bank free size (f32)
KC = C // P        # k-chunks
DC = D // P        # d-chunks
TC = T // TN       # t-chunks

_CACHE = {}

# test.py reads LAST_RESULTS for exec_time_ns when tracing is enabled.
LAST_RESULTS = None


def _build():
    nc = bacc.Bacc(
        "TRN2", target_bir_lowering=False, debug=False, enable_asserts=True,
        num_devices=N_CORES,
    )
    x = nc.dram_tensor("x", [BS, C, T], F32, kind="ExternalInput").ap()
    w = nc.dram_tensor("w", [BS, C, D], F32, kind="ExternalInput").ap()
    y = nc.dram_tensor("y", [BS, D, T], F32, kind="ExternalOutput").ap()

    with tile.TileContext(nc) as tc:
        with tc.tile_pool(name="xp", bufs=4) as xp, \
             tc.tile_pool(name="wp", bufs=4) as wp, \
             tc.tile_pool(name="yp", bufs=4) as yp, \
             tc.tile_pool(name="pp", bufs=8, space="PSUM") as pp:
            for b in range(BS):
                xk = []
                wk = []
                for k in range(KC):
                    xt = xp.tile([P, T], F32, name=f"x_{b}_{k}", tag="x")
                    nc.sync.dma_start(xt[:], x[b, k * P:(k + 1) * P, :])
                    xk.append(xt)
                    wt = wp.tile([P, D], F32, name=f"w_{b}_{k}", tag="w")
                    nc.sync.dma_start(wt[:], w[b, k * P:(k + 1) * P, :])
                    wk.append(wt)
                for d in range(DC):
                    yt = yp.tile([P, T], F32, name=f"y_{b}_{d}", tag="y")
                    for t in range(TC):
                        ps = pp.tile([P, TN], F32, name=f"ps_{b}_{d}_{t}", tag="ps")
                        for k in range(KC):
                            nc.tensor.matmul(
                                ps[:],
                                wk[k][:, d * P:(d + 1) * P],
                                xk[k][:, t * TN:(t + 1) * TN],
                                start=(k == 0), stop=(k == KC - 1),
                            )
                        nc.vector.tensor_copy(yt[:, t * TN:(t + 1) * TN], ps[:])
                    nc.sync.dma_start(y[b, d * P:(d + 1) * P, :], yt[:])
    nc.compile()
    return nc


def kernel(x, weights, condition):
    global LAST_RESULTS
    x = np.ascontiguousarray(np.asarray(x, dtype=np.float32))
    weights = np.ascontiguousarray(np.asarray(weights, dtype=np.float32))
    condition = np.asarray(condition).astype(np.int64)
    assert x.shape == (B, C, T) and weights.shape[1:] == (C, D)
    assert condition.shape == (B,)

    if "nc" not in _CACHE:
        _CACHE["nc"] = _build()
    nc = _CACHE["nc"]

    w_gathered = np.ascontiguousarray(weights[condition])  # [B, C, D]
    in_maps = [
        {
            "x": np.ascontiguousarray(x[i * BS:(i + 1) * BS]),
            "w": np.ascontiguousarray(w_gathered[i * BS:(i + 1) * BS]),
        }
        for i in range(N_CORES)
    ]
    res = run_bass_kernel_spmd(nc, in_maps, core_ids=list(range(N_CORES)))
    LAST_RESULTS = res
    return np.concatenate([r["y"] for r in res.results], axis=0)


# revision 2
# speedup vs baseline: 1.1207x; 1.1207x over previous
"""Conditional-layers (MoE routing) kernel for Trainium2, 8 NeuronCores.
fp16 hi/lo split variant: fp32 matmul emulated by 3 fp16 matmuls (PE runs
fp16 at 1 cycle/row vs fp32's 4), with inputs decomposed host-side.

Reference: out[b] = W[condition[b]].T @ x[b]  (einsum 'bct,bcd->bdt')

Decomposition (host):
  x   = xhi + xlo + O(2^-22 x)     xhi = f16(x), xlo = f16(x - xhi)
  256 w = whi + wlo + O(2^-22)     (x256 keeps wlo out of f16 subnormals)
Device:  psum = whi'xhi + whi'xlo + wlo'xhi  (per k-chunk, PSUM-accumulated)
         y = psum / 256            (exact power-of-2 scale on the copyback)
Dropped wlo'xlo term is O(2^-22) relative — fp32-rounding-level accuracy.
hi/lo are interleaved host-side along the free axis (xcat[.., :T]=hi,
[.., T:]=lo) so each (b, k-chunk) loads as ONE contiguous 2MB DMA.

Sharding: data-parallel over batch, 8 samples per core; per-sample weight
gather (the routing) is host-side so the compiled program is
condition-independent.
"""

import numpy as np

import concourse.mybir as mybir
import concourse.tile as tile
from concourse import bacc
from concourse.bass_utils import run_bass_kernel_spmd

F32 = mybir.dt.float32
F16 = mybir.dt.float16

N_CORES = 8
B = 64
BS = B // N_CORES
C = 256
D = 256
T = 4096
P = 128
TN = 512
KC = C // P
DC = D // P
TC = T // TN

W_SCALE = 256.0

_CACHE = {}
LAST_RESULTS = None


def _build():
    nc = bacc.Bacc(
        "TRN2", target_bir_lowering=False, debug=False, enable_asserts=True,
        num_devices=N_CORES,
    )
    # hi/lo interleaved along the last axis: [..., :T]=hi, [..., T:]=lo.
    xc = nc.dram_tensor("xc", [BS, C, 2 * T], F16, kind="ExternalInput").ap()
    wc = nc.dram_tensor("wc", [BS, C, 2 * D], F16, kind="ExternalInput").ap()
    y = nc.dram_tensor("y", [BS, D, T], F32, kind="ExternalOutput").ap()

    with tile.TileContext(nc) as tc:
        with tc.tile_pool(name="xp", bufs=6) as xp, \
             tc.tile_pool(name="wp", bufs=6) as wp, \
             tc.tile_pool(name="yp", bufs=4) as yp, \
             tc.tile_pool(name="pp", bufs=8, space="PSUM") as pp:
            for b in range(BS):
                # FIFO load ring: tiny w tables first, then hi planes (all
                # the first accumulation terms need), then lo planes.
                xk = []
                wk = []
                for k in range(KC):
                    ks = slice(k * P, (k + 1) * P)
                    wt = wp.tile([P, 2 * D], F16, name=f"w_{b}_{k}", tag="w")
                    nc.sync.dma_start(wt[:], wc[b, ks, :])
                    wk.append(wt)
                    xt = xp.tile([P, 2 * T], F16, name=f"x_{b}_{k}", tag="x")
                    xk.append(xt)
                for k in range(KC):
                    ks = slice(k * P, (k + 1) * P)
                    nc.sync.dma_start(xk[k][:, :T], xc[b, ks, :T])
                for k in range(KC):
                    ks = slice(k * P, (k + 1) * P)
                    nc.sync.dma_start(xk[k][:, T:], xc[b, ks, T:])
                for d in range(DC):
                    ds = slice(d * P, (d + 1) * P)
                    yt = yp.tile([P, T], F32, name=f"y_{b}_{d}", tag="y")
                    for t in range(TC):
                        ts = slice(t * TN, (t + 1) * TN)
                        ps = pp.tile([P, TN], F32, name=f"ps_{b}_{d}_{t}", tag="ps")
                        terms = []
                        for k in range(KC):
                            xh = xk[k][:, t * TN:(t + 1) * TN]
                            xl = xk[k][:, T + t * TN:T + (t + 1) * TN]
                            wh = wk[k][:, d * P:(d + 1) * P]
                            wl = wk[k][:, D + d * P:D + (d + 1) * P]
                            terms += [(wh, xh), (wh, xl), (wl, xh)]
                        for i, (wt_, xt_) in enumerate(terms):
                            nc.tensor.matmul(
                                ps[:], wt_, xt_,
                                start=(i == 0), stop=(i == len(terms) - 1),
                            )
                        # Unscale on the copyback; alternate engines so
                        # neither DVE nor ACT becomes a serial bottleneck.
                        if t % 2 == 0:
                            nc.vector.tensor_scalar_mul(
                                yt[:, ts], ps[:], 1.0 / W_SCALE)
                        else:
                            nc.scalar.mul(yt[:, ts], ps[:], 1.0 / W_SCALE)
                        # Stores ride the ACT HW-DGE ring (qActDynamicHW) so
                        # they don't FIFO-queue behind loads on the sync ring;
                        # drain each 1/4-row as soon as its two copies land.
                        if t % 2 == 1:
                            qs = slice((t - 1) * TN, (t + 1) * TN)
                            nc.scalar.dma_start(y[b, ds, qs], yt[:, qs])
    nc.compile()
    return nc


def kernel(x, weights, condition):
    global LAST_RESULTS
    x = np.ascontiguousarray(np.asarray(x, dtype=np.float32))
    weights = np.ascontiguousarray(np.asarray(weights, dtype=np.float32))
    condition = np.asarray(condition).astype(np.int64)
    assert x.shape == (B, C, T) and weights.shape[1:] == (C, D)
    assert condition.shape == (B,)

    if "nc" not in _CACHE:
        _CACHE["nc"] = _build()
    nc = _CACHE["nc"]

    xc = np.empty((B, C, 2 * T), dtype=np.float16)
    xhi = x.astype(np.float16)
    xc[:, :, :T] = xhi
    xc[:, :, T:] = (x - xhi.astype(np.float32)).astype(np.float16)

    ws = (weights[condition] * np.float32(W_SCALE)).astype(np.float32)  # [B,C,D]
    wc = np.empty((B, C, 2 * D), dtype=np.float16)
    whi = ws.astype(np.float16)
    wc[:, :, :D] = whi
    wc[:, :, D:] = (ws - whi.astype(np.float32)).astype(np.float16)

    in_maps = [
        {
            "xc": np.ascontiguousarray(xc[i * BS:(i + 1) * BS]),
            "wc": np.ascontiguousarray(wc[i * BS:(i + 1) * BS]),
        }
        for i in range(N_CORES)
    ]
    # Device faults (NRT_EXEC_UNIT_UNRECOVERABLE) are rare and transient on
    # this fabric — retry a couple of times before giving up.
    last_exc = None
    for _ in range(3):
        try:
            res = run_bass_kernel_spmd(nc, in_maps, core_ids=list(range(N_CORES)))
            break
        except Exception as e:  # noqa: BLE001
            last_exc = e
            import time
            time.sleep(5)
    else:
        raise last_exc
    LAST_RESULTS = res
    return np.concatenate([r["y"] for r in res.results], axis=0)
